# revision 25
# baseline (speedup 1.0000x reference)
"""Block-sparse attention TRN2 kernel (8 NeuronCores, SPMD over batch*heads).

Contract: kernel(**inputs) takes FULL unsharded inputs
  query/key/value: (2, 16, 2048, 128) f32, block_mask: (16, 16) bool,
  block_size: 128
and returns the FULL (2, 16, 2048, 128) f32 output.

Math per (b, h): for each 128x128 block pair (i, j) with block_mask[i, j]:
  A_ij = softmax(Q_i K_j^T / sqrt(128)) (softmax per block row, no
  cross-block merge), O_i = sum_j A_ij V_j.

Device layout ([k, q] orientation so no on-chip transposes are needed):
  For key block j, scores for the active query blocks are packed into
  512-col (one PSUM bank) chunks: S^T = matmul(lhsT=KT[:, j], rhs=QT runs)
  in f16 (full-rate). exp on ACT (PSUM f32 -> SBUF f16).

  Denominators are placed COMPACTLY on PSUM partitions via a shifted
  one-hot stationary: T[128, 257] is all zeros except column 128 (ones).
  matmul(d_grp, lhsT=T[:, 128-k:256-k], rhs=E_chunk) adds E's column
  sums to partition row k and +0 everywhere else, so a group of up to 7
  chunks accumulates its denominators into rows 0..6 of one PSUM bank.
  One DVE reciprocal_approx_fast per group then covers all of the
  group's 3.5k denominators at once (vs one [128,512] recip per chunk),
  r is converted to f16, bounced through a DRAM scratch, and DMA'd back
  with a stride-0 source AP that replicates it across all 128
  partitions. The normalize multiply is then a single all-SBUF all-f16
  DVE tensor_tensor per group (4x DVE perf mode).

  O^T += V_j^T.T @ Ahat^T accumulates in PSUM over j; drained via
  GPSIMD copy (f32->f16) + DMA per bank. Emission is software-pipelined
  A(g); B(g); C(g-2) so the PE never waits on the r round-trip.
"""

import math
from collections import deque

import numpy as np

B, H, S, D = 2, 16, 2048, 128
BS = 128
NB = S // BS
N_CORES = 8
N_HEADS = B * H
HPC = N_HEADS // N_CORES  # heads per core
CH = 512  # chunk columns = one PSUM bank of f32
GMAX = 7  # chunks per denominator group (<= 128 rows of one PSUM bank)
SCALE = 1.0 / math.sqrt(float(D))


def _plan(mask):
    """Mask-derived emission plan (shared by every head/core).

    Returns a flat chunk schedule; each chunk is (used, mm1s, pieces) with
      mm1s   = (off_in_chunk, [qoff, ...], width, j); two qoffs means a
               paired single-block matmul via a 3-level access pattern.
      pieces = (q_out_col, width, off_in_chunk, j) MM2 pieces, split at
               output PSUM bank boundaries and first-touch flips.
    Partial tail chunks are merged ACROSS key blocks j (exp/denominator/
    normalize are j-agnostic; all KT/V slices are SBUF-resident).
    """
    mask = np.asarray(mask).astype(bool)
    assert mask.shape == (NB, NB)
    cap = CH // BS  # blocks per chunk

    groups = []  # (j, [(i0, ln), ...]) per finalized bin, emission order
    pending = []
    pend_fill = 0

    def flush():
        nonlocal pend_fill
        if pending:
            groups.append(list(pending))
            pending.clear()
            pend_fill = 0

    for j in range(NB):
        act = [i for i in range(NB) if mask[i, j]]
        runs = []
        for i in act:
            if runs and runs[-1][0] + runs[-1][1] == i:
                runs[-1][1] += 1
            else:
                runs.append([i, 1])
        items = []
        for i0, ln in runs:
            while ln > cap:
                items.append((i0, cap))
                i0 += cap
                ln -= cap
            items.append((i0, ln))
        bins = []
        for i0, ln in sorted(items, key=lambda x: -x[1]):
            for b in bins:
                if b[0] + ln <= cap:
                    b[0] += ln
                    b[1].append((i0, ln))
                    break
            else:
                bins.append([ln, [(i0, ln)]])
        for fill, bitems in bins:
            if fill == cap:
                groups.append([(j, it) for it in sorted(bitems)])
            else:
                if pend_fill + fill > cap:
                    flush()
                pending.extend((j, it) for it in sorted(bitems))
                pend_fill += fill
    flush()

    sched = []
    bank_counts = [0] * (S // CH)
    touched = set()
    for gitems in groups:
        byj = {}
        for j, it in gitems:
            byj.setdefault(j, []).append(it)
        placed = []
        mm1s = []
        off = 0
        for j in sorted(byj):
            jitems = byj[j]
            longs = sorted([it for it in jitems if it[1] > 1])
            singles = sorted([it for it in jitems if it[1] == 1])
            sing_offs = []
            for i0, ln in longs + singles:
                placed.append((off, i0 * BS, ln * BS, j))
                if ln > 1:
                    mm1s.append((off, [i0 * BS], ln * BS, j))
                else:
                    sing_offs.append((off, i0 * BS))
                off += ln * BS
            for k in range(0, len(sing_offs) - 1, 2):
                mm1s.append(
                    (sing_offs[k][0], [sing_offs[k][1], sing_offs[k + 1][1]],
                     2 * BS, j)
                )
            if len(sing_offs) % 2:
                mm1s.append((sing_offs[-1][0], [sing_offs[-1][1]], BS, j))
        used = off
        # PSUM start=True marks its whole zero region pending; each
        # later matmul must touch a uniformly pending (overwrite) or
        # uniformly cleared (accumulate) range, so pieces split at both
        # bank boundaries and first-touch flips.
        pieces = []
        for o, qoff, w, j in placed:
            ib0 = qoff // BS
            nblk = w // BS
            blk = 0
            while blk < nblk:
                ib = ib0 + blk
                ft = ib not in touched
                bank = (ib * BS) // CH
                end = blk + 1
                while end < nblk:
                    ib2 = ib0 + end
                    if (ib2 not in touched) != ft or (ib2 * BS) // CH != bank:
                        break
                    end += 1
                for b2 in range(blk, end):
                    touched.add(ib0 + b2)
                qo = ib * BS
                wp = (end - blk) * BS
                pieces.append((qo, wp, o + (qo - qoff), j))
                bank_counts[bank] += 1
                blk = end
        sched.append((used, mm1s, pieces))
    empty_rows = [i for i in range(NB) if not mask[i].any()]
    return sched, bank_counts, empty_rows


def _split_groups(n, gmax=GMAX):
    """Split n chunks into balanced groups of size <= gmax."""
    ng = (n + gmax - 1) // gmax
    base = n // ng
    rem = n % ng
    sizes = [base + (1 if i < rem else 0) for i in range(ng)]
    out = []
    at = 0
    for s in sizes:
        out.append((at, s))
        at += s
    return out


def _build(mask):
    import concourse.bass as bass
    import concourse.bacc as bacc
    import concourse.tile as tile
    from concourse import mybir

    f32 = mybir.dt.float32
    f16 = mybir.dt.float16
    AF = mybir.ActivationFunctionType

    sched, bank_counts, empty_rows = _plan(mask)
    nch = len(sched)
    grps = _split_groups(nch)
    rcols_per_head = nch * CH

    nc = bacc.Bacc(
        "TRN2",
        target_bir_lowering=False,
        debug=False,
        enable_asserts=False,
        num_devices=N_CORES,
    )
    qt_d = nc.dram_tensor("qt", (HPC, D, S), f16, kind="ExternalInput").ap()
    kt_d = nc.dram_tensor("kt", (HPC, D, S), f16, kind="ExternalInput").ap()
    v_d = nc.dram_tensor("v", (HPC, BS, NB * BS), f16, kind="ExternalInput").ap()
    ot_d = nc.dram_tensor("ot", (HPC, D, S), f16, kind="ExternalOutput").ap()
    r_dram = nc.dram_tensor(
        "rscratch", (2, rcols_per_head), f16, kind="Internal"
    ).ap()

    with tile.TileContext(nc) as tc:
        with (
            tc.tile_pool(name="heads", bufs=2) as heads,
            tc.tile_pool(name="const", bufs=1) as const,
            tc.tile_pool(name="e", bufs=7) as epool,
            tc.tile_pool(name="eh", bufs=3) as ehpool,
            tc.tile_pool(name="rrep", bufs=6) as rreppool,
            tc.tile_pool(name="rsm", bufs=6) as rsmpool,
            tc.tile_pool(name="outp", bufs=3) as outpool,
            tc.tile_pool(name="ps_s", bufs=2, space="PSUM") as ps_s,
            tc.tile_pool(name="ps_d", bufs=2, space="PSUM") as ps_d,
            tc.tile_pool(name="ps_o", bufs=1, space="PSUM") as ps_o,
        ):
            # Shifted one-hot stationary: zeros except column GMAX-1 =
            # ones. The width-128 slice starting at GMAX-1-k has its
            # column k all-ones and every other column zero.
            T = const.tile([128, 128 + GMAX - 1], f16)
            nc.vector.memset(T[:], 0.0)
            nc.vector.memset(T[:, GMAX - 1 : GMAX], 1.0)

            # Per-head persistent state while its groups are in flight.
            hstate = {}

            def load_head(h):
                # Input loads ride the gpsimd queue so the latency-
                # critical r broadcasts on the sync queue never sit
                # behind a 0.5MB head load. Head 0 is the warmup
                # critical path: split its loads into column slices
                # across three queues so the first MM1s can start as
                # soon as their slices land.
                qt_t = heads.tile([D, S], f16, tag="qt")
                kt_t = heads.tile([D, S], f16, tag="kt")
                v_t = heads.tile([BS, NB * BS], f16, tag="v")
                if h == 0:
                    # qt FIRST: the first chunk's MM1s read query runs
                    # scattered across the whole sequence, so they gate
                    # on the LAST qt slice; kt is consumed in ascending
                    # j order so kt slice 0 suffices to start. v is only
                    # needed by the first MM2s (~15us in).
                    engs = [nc.sync, nc.gpsimd]
                    n = 0
                    for t_, d_ in ((qt_t, qt_d), (kt_t, kt_d), (v_t, v_d)):
                        for c in range(4):
                            sl = slice(c * CH, (c + 1) * CH)
                            engs[n % 2].dma_start(
                                out=t_[:, sl], in_=d_[h][:, sl]
                            )
                            n += 1
                else:
                    nc.gpsimd.dma_start(out=qt_t[:], in_=qt_d[h])
                    nc.gpsimd.dma_start(out=kt_t[:], in_=kt_d[h])
                    nc.gpsimd.dma_start(out=v_t[:], in_=v_d[h])
                return (qt_t, kt_t, v_t)

            # Flat list of group work items across heads.
            work = []  # (h, g_idx, c0, ng)
            for h in range(HPC):
                for g_idx, (c0, ng) in enumerate(grps):
                    work.append((h, g_idx, c0, ng))

            inflight = {}  # slot -> dict with tiles for phase C

            def phase_a_b(slot, h, g_idx, c0, ng):
                st = hstate[h]
                qt_t, kt_t, v_t = st["io"]
                e_grp = epool.tile([BS, GMAX * CH], f16, tag="e")
                d_grp = ps_d.tile([128, CH], f32, tag="d")

                def emit_denom(k):
                    # Emitted one chunk late so the PE does not head-of-
                    # line block on exp(k) right after MM1(k).
                    nc.tensor.matmul(
                        d_grp[:, :CH],
                        lhsT=T[:, GMAX - 1 - k : GMAX - 1 - k + 128],
                        rhs=e_grp[:, k * CH : (k + 1) * CH],
                        start=(k == 0),
                        stop=(k == ng - 1),
                    )

                for k in range(ng):
                    used, mm1s, _ = sched[c0 + k]
                    s_ps = ps_s.tile([BS, CH], f32, tag="s")
                    for idx, (off, qoffs, w, j) in enumerate(mm1s):
                        if len(qoffs) == 2:
                            base = qt_t[:, qoffs[0] : qoffs[0] + BS]
                            rhs = bass.AP(
                                tensor=base.tensor,
                                offset=base.offset,
                                ap=[
                                    base.ap[0],
                                    [qoffs[1] - qoffs[0], 2],
                                    [1, BS],
                                ],
                            )
                        else:
                            rhs = qt_t[:, qoffs[0] : qoffs[0] + w]
                        nc.tensor.matmul(
                            s_ps[:, off : off + w],
                            lhsT=kt_t[:, j * BS : (j + 1) * BS],
                            rhs=rhs,
                            start=(idx == 0),
                            stop=(idx == len(mm1s) - 1),
                        )
                    eb = k * CH
                    nc.scalar.activation(
                        e_grp[:, eb : eb + used], s_ps[:, :used], AF.Exp,
                        scale=SCALE,
                    )
                    # Pad partial chunks so every denominator in rows
                    # [:ng] is positive (recip of 0 would trip the
                    # non-finite guard) and the start=True matmul covers
                    # the full bank.
                    if used < CH:
                        nc.gpsimd.memset(e_grp[:, eb + used : eb + CH], 1.0)
                    if k >= 1:
                        emit_denom(k - 1)
                emit_denom(ng - 1)
                # Phase B: batched reciprocal -> f16 -> DRAM -> broadcast.
                r_f32 = rsmpool.tile([GMAX, CH], f32, tag="rf32")
                nc.vector.reciprocal_approx_fast(
                    r_f32[:ng, :], d_grp[:ng, :]
                )
                r_f16 = rsmpool.tile([GMAX, CH], f16, tag="rf16")
                nc.vector.tensor_scalar_mul(r_f16[:ng, :], r_f32[:ng, :], 1.0)
                roff = (h % 2) * rcols_per_head + c0 * CH
                nc.gpsimd.dma_start(
                    out=bass.AP(
                        tensor=r_dram.tensor, offset=roff, ap=[[1, ng * CH]]
                    ),
                    in_=r_f16[:ng, :],
                )
                r_rep = rreppool.tile([BS, GMAX * CH], f16, tag="rr")
                # The sync queue is dedicated to these latency-critical
                # broadcast reads.
                nc.sync.dma_start(
                    out=r_rep[:, : ng * CH],
                    in_=bass.AP(
                        tensor=r_dram.tensor,
                        offset=roff,
                        ap=[[0, 128], [1, ng * CH]],
                    ),
                )
                inflight[slot] = {
                    "h": h, "c0": c0, "ng": ng,
                    "e": e_grp, "rr": r_rep, "v": v_t,
                }

            def flush_drains(st, h):
                for bk in st["pending_drains"]:
                    o_sb = outpool.tile([D, CH], f16, tag="osb")
                    nc.vector.tensor_scalar_mul(
                        o_sb[:], st["o_ps"][:, bk * CH : (bk + 1) * CH], 1.0
                    )
                    nc.gpsimd.dma_start(
                        out=ot_d[h, :, bk * CH : (bk + 1) * CH], in_=o_sb[:]
                    )
                st["pending_drains"] = []

            def phase_c(slot):
                w = inflight.pop(slot, None)
                if w is None:
                    return
                h = w["h"]
                st = hstate[h]
                flush_drains(st, h)
                if not st["zeroed"]:
                    # Zero never-touched query blocks here (not at head
                    # start) so the memsets are emitted AFTER the
                    # previous head's drains on the same DVE queue.
                    for i in empty_rows:
                        nc.vector.memset(
                            st["o_ps"][:, i * BS : (i + 1) * BS], 0.0
                        )
                    st["zeroed"] = True
                e_grp, r_rep, v_t = w["e"], w["rr"], w["v"]
                ng, c0 = w["ng"], w["c0"]
                eh_grp = ehpool.tile([BS, GMAX * CH], f16, tag="eh")
                # Two-half normalize so the first MM2s start sooner.
                hcols = ((ng + 1) // 2) * CH
                for lo, hi in ((0, hcols), (hcols, ng * CH)):
                    if hi > lo:
                        nc.vector.tensor_tensor(
                            out=eh_grp[:, lo:hi],
                            in0=e_grp[:, lo:hi],
                            in1=r_rep[:, lo:hi],
                            op=mybir.AluOpType.mult,
                        )
                for k in range(ng):
                    _, _, pieces = sched[c0 + k]
                    for qo, wp, op, j in pieces:
                        bk = qo // CH
                        first = bk not in st["started"]
                        st["started"].add(bk)
                        st["remaining"][bk] -= 1
                        done = st["remaining"][bk] == 0
                        nc.tensor.matmul(
                            st["o_ps"][:, qo : qo + wp],
                            lhsT=v_t[:, j * BS : (j + 1) * BS],
                            rhs=eh_grp[:, k * CH + op : k * CH + op + wp],
                            start=first,
                            stop=done,
                        )
                        if done:
                            if h == HPC - 1 and st["left"] == 1:
                                # Final group of the last head: nothing
                                # runs after it, so drain immediately to
                                # overlap the DVE copies + DMA with the
                                # remaining MM2s instead of serializing
                                # the whole 4-bank chain at the end.
                                st["pending_drains"].append(bk)
                                flush_drains(st, h)
                            else:
                                # Defer the drain one slot so its PSUM
                                # read does not block the DVE wait queue
                                # while this bank's MM2s are in flight.
                                st["pending_drains"].append(bk)
                st["left"] -= 1
                if st["left"] == 0:
                    # Banks no piece ever touched (fully masked-off
                    # output) drain the empty-row memset zeros.
                    for bk in range(S // CH):
                        if bank_counts[bk] == 0:
                            st["pending_drains"].append(bk)
                    flush_drains(st, h)
                    del hstate[h]

            LAG = 4
            nwork = len(work)
            c_next = 0  # next phase-C slot to consume, in order
            for slot, (h, g_idx, c0, ng) in enumerate(work):
                # Progressively shorter phase-C lag near the end of the
                # work list (4 -> 2) so the pipeline drains during the
                # last head's phase A/B instead of entirely after it.
                # Lag 2 still gives the r DRAM round-trip a full A/B
                # slot (~5us) of cover.
                lag = min(LAG, max(2, nwork - 1 - slot))
                while c_next <= slot - lag:
                    phase_c(c_next)
                    c_next += 1
                if g_idx == 0:
                    o_ps = ps_o.tile([D, S], f32, tag="o")
                    hstate[h] = {
                        "io": load_head(h),
                        "o_ps": o_ps,
                        "started": set(),
                        "remaining": list(bank_counts),
                        "left": len(grps),
                        "zeroed": False,
                        "pending_drains": [],
                    }
                phase_a_b(slot, h, g_idx, c0, ng)
            while c_next < nwork:
                phase_c(c_next)
                c_next += 1

    nc.finalize()
    return nc
_CACHE = {}


def _get_program(mask):
    key = np.asarray(mask).astype(bool).tobytes()
    if key not in _CACHE:
        _CACHE[key] = _build(mask)
    return _CACHE[key]


def _shard_inputs(query, key, value):
    q = np.ascontiguousarray(query, dtype=np.float32).reshape(N_HEADS, S, D)
    k = np.ascontiguousarray(key, dtype=np.float32).reshape(N_HEADS, S, D)
    v = np.ascontiguousarray(value, dtype=np.float32).reshape(N_HEADS, S, D)
    qt = np.ascontiguousarray(q.transpose(0, 2, 1).astype(np.float16))
    kt = np.ascontiguousarray(k.transpose(0, 2, 1).astype(np.float16))
    v16 = np.ascontiguousarray(
        v.reshape(N_HEADS, NB, BS, D).transpose(0, 2, 1, 3).astype(np.float16)
    ).reshape(N_HEADS, BS, NB * BS)
    in_maps = []
    for c in range(N_CORES):
        sl = slice(c * HPC, (c + 1) * HPC)
        in_maps.append(
            {
                "qt": np.ascontiguousarray(qt[sl]),
                "kt": np.ascontiguousarray(kt[sl]),
                "v": np.ascontiguousarray(v16[sl]),
            }
        )
    return in_maps


def _unshard_output(results):
    ot = np.concatenate([r["ot"] for r in results], axis=0)  # (32, D, S) f16
    out = ot.transpose(0, 2, 1).reshape(B, H, S, D)
    return np.ascontiguousarray(out, dtype=np.float32)


def kernel(query, key, value, block_mask, block_size, _trace=False):
    from concourse.bass_utils import run_bass_kernel_spmd

    assert int(block_size) == BS
    nc = _get_program(block_mask)
    in_maps = _shard_inputs(query, key, value)
    res = run_bass_kernel_spmd(nc, in_maps, core_ids=list(range(N_CORES)), trace=_trace)
    out = _unshard_output(res.results)
    if _trace:
        return out, res
    return out



# revision 27
# speedup vs baseline: 1.0461x; 1.0461x over previous
"""Block-sparse attention TRN2 kernel (8 NeuronCores, SPMD over batch*heads).

Contract: kernel(**inputs) takes FULL unsharded inputs
  query/key/value: (2, 16, 2048, 128) f32, block_mask: (16, 16) bool,
  block_size: 128
and returns the FULL (2, 16, 2048, 128) f32 output.

Math per (b, h): for each 128x128 block pair (i, j) with block_mask[i, j]:
  A_ij = softmax(Q_i K_j^T / sqrt(128)) (softmax per block row, no
  cross-block merge), O_i = sum_j A_ij V_j.

Device layout ([k, q] orientation so no on-chip transposes are needed):
  For key block j, scores for the active query blocks are packed into
  512-col (one PSUM bank) chunks: S^T = matmul(lhsT=KT[:, j], rhs=QT runs)
  in f16 (full-rate). exp on ACT (PSUM f32 -> SBUF f16).

  Denominators are placed COMPACTLY on PSUM partitions via a shifted
  one-hot stationary: T[128, 257] is all zeros except column 128 (ones).
  matmul(d_grp, lhsT=T[:, 128-k:256-k], rhs=E_chunk) adds E's column
  sums to partition row k and +0 everywhere else, so a group of up to 7
  chunks accumulates its denominators into rows 0..6 of one PSUM bank.
  One DVE reciprocal_approx_fast per group then covers all of the
  group's 3.5k denominators at once (vs one [128,512] recip per chunk),
  r is converted to f16, bounced through a DRAM scratch, and DMA'd back
  with a stride-0 source AP that replicates it across all 128
  partitions. The normalize multiply is then a single all-SBUF all-f16
  DVE tensor_tensor per group (4x DVE perf mode).

  O^T += V_j^T.T @ Ahat^T accumulates in PSUM over j; drained via
  GPSIMD copy (f32->f16) + DMA per bank. Emission is software-pipelined
  A(g); B(g); C(g-2) so the PE never waits on the r round-trip.
"""

import math
from collections import deque

import numpy as np

B, H, S, D = 2, 16, 2048, 128
BS = 128
NB = S // BS
N_CORES = 8
N_HEADS = B * H
HPC = N_HEADS // N_CORES  # heads per core
CH = 512  # chunk columns = one PSUM bank of f32
GMAX = 7  # chunks per denominator group (<= 128 rows of one PSUM bank)
SCALE = 1.0 / math.sqrt(float(D))


def _plan(mask):
    """Mask-derived emission plan (shared by every head/core).

    Returns a flat chunk schedule; each chunk is (used, mm1s, pieces) with
      mm1s   = (off_in_chunk, [qoff, ...], width, j); two qoffs means a
               paired single-block matmul via a 3-level access pattern.
      pieces = (q_out_col, width, off_in_chunk, j) MM2 pieces, split at
               output PSUM bank boundaries and first-touch flips.
    Partial tail chunks are merged ACROSS key blocks j (exp/denominator/
    normalize are j-agnostic; all KT/V slices are SBUF-resident).
    """
    mask = np.asarray(mask).astype(bool)
    assert mask.shape == (NB, NB)
    cap = CH // BS  # blocks per chunk

    groups = []  # (j, [(i0, ln), ...]) per finalized bin, emission order
    pending = []
    pend_fill = 0

    def flush():
        nonlocal pend_fill
        if pending:
            groups.append(list(pending))
            pending.clear()
            pend_fill = 0

    for j in range(NB):
        act = [i for i in range(NB) if mask[i, j]]
        runs = []
        for i in act:
            if runs and runs[-1][0] + runs[-1][1] == i:
                runs[-1][1] += 1
            else:
                runs.append([i, 1])
        items = []
        for i0, ln in runs:
            while ln > cap:
                items.append((i0, cap))
                i0 += cap
                ln -= cap
            items.append((i0, ln))
        bins = []
        for i0, ln in sorted(items, key=lambda x: -x[1]):
            for b in bins:
                if b[0] + ln <= cap:
                    b[0] += ln
                    b[1].append((i0, ln))
                    break
            else:
                bins.append([ln, [(i0, ln)]])
        for fill, bitems in bins:
            if fill == cap:
                groups.append([(j, it) for it in sorted(bitems)])
            else:
                if pend_fill + fill > cap:
                    flush()
                pending.extend((j, it) for it in sorted(bitems))
                pend_fill += fill
    flush()

    sched = []
    bank_counts = [0] * (S // CH)
    touched = set()
    for gitems in groups:
        byj = {}
        for j, it in gitems:
            byj.setdefault(j, []).append(it)
        placed = []
        mm1s = []
        off = 0
        for j in sorted(byj):
            jitems = byj[j]
            longs = sorted([it for it in jitems if it[1] > 1])
            singles = sorted([it for it in jitems if it[1] == 1])
            sing_offs = []
            for i0, ln in longs + singles:
                placed.append((off, i0 * BS, ln * BS, j))
                if ln > 1:
                    mm1s.append((off, [i0 * BS], ln * BS, j))
                else:
                    sing_offs.append((off, i0 * BS))
                off += ln * BS
            for k in range(0, len(sing_offs) - 1, 2):
                mm1s.append(
                    (sing_offs[k][0], [sing_offs[k][1], sing_offs[k + 1][1]],
                     2 * BS, j)
                )
            if len(sing_offs) % 2:
                mm1s.append((sing_offs[-1][0], [sing_offs[-1][1]], BS, j))
        used = off
        # PSUM start=True marks its whole zero region pending; each
        # later matmul must touch a uniformly pending (overwrite) or
        # uniformly cleared (accumulate) range, so pieces split at both
        # bank boundaries and first-touch flips.
        pieces = []
        for o, qoff, w, j in placed:
            ib0 = qoff // BS
            nblk = w // BS
            blk = 0
            while blk < nblk:
                ib = ib0 + blk
                ft = ib not in touched
                bank = (ib * BS) // CH
                end = blk + 1
                while end < nblk:
                    ib2 = ib0 + end
                    if (ib2 not in touched) != ft or (ib2 * BS) // CH != bank:
                        break
                    end += 1
                for b2 in range(blk, end):
                    touched.add(ib0 + b2)
                qo = ib * BS
                wp = (end - blk) * BS
                pieces.append((qo, wp, o + (qo - qoff), j))
                bank_counts[bank] += 1
                blk = end
        sched.append((used, mm1s, pieces))
    empty_rows = [i for i in range(NB) if not mask[i].any()]
    return sched, bank_counts, empty_rows


def _split_groups(n, gmax=GMAX):
    """Split n chunks into balanced groups of size <= gmax."""
    ng = (n + gmax - 1) // gmax
    base = n // ng
    rem = n % ng
    sizes = [base + (1 if i < rem else 0) for i in range(ng)]
    out = []
    at = 0
    for s in sizes:
        out.append((at, s))
        at += s
    return out


def _build(mask):
    import concourse.bass as bass
    import concourse.bacc as bacc
    import concourse.tile as tile
    from concourse import mybir

    f32 = mybir.dt.float32
    f16 = mybir.dt.float16
    AF = mybir.ActivationFunctionType

    sched, bank_counts, empty_rows = _plan(mask)
    nch = len(sched)
    grps = _split_groups(nch)
    rcols_per_head = nch * CH

    nc = bacc.Bacc(
        "TRN2",
        target_bir_lowering=False,
        debug=False,
        enable_asserts=False,
        num_devices=N_CORES,
    )
    qt_d = nc.dram_tensor("qt", (HPC, D, S), f16, kind="ExternalInput").ap()
    kt_d = nc.dram_tensor("kt", (HPC, D, S), f16, kind="ExternalInput").ap()
    v_d = nc.dram_tensor("v", (HPC, BS, NB * BS), f16, kind="ExternalInput").ap()
    ot_d = nc.dram_tensor("ot", (HPC, D, S), f16, kind="ExternalOutput").ap()
    r_dram = nc.dram_tensor(
        "rscratch", (2, rcols_per_head), f16, kind="Internal"
    ).ap()

    with tile.TileContext(nc) as tc:
        with (
            tc.tile_pool(name="heads", bufs=2) as heads,
            tc.tile_pool(name="const", bufs=1) as const,
            tc.tile_pool(name="e", bufs=7) as epool,
            tc.tile_pool(name="eh", bufs=3) as ehpool,
            tc.tile_pool(name="rrep", bufs=6) as rreppool,
            tc.tile_pool(name="rsm", bufs=6) as rsmpool,
            tc.tile_pool(name="outp", bufs=3) as outpool,
            tc.tile_pool(name="ps_s", bufs=2, space="PSUM") as ps_s,
            tc.tile_pool(name="ps_d", bufs=2, space="PSUM") as ps_d,
            tc.tile_pool(name="ps_o", bufs=1, space="PSUM") as ps_o,
        ):
            # Shifted one-hot stationary: zeros except column GMAX-1 =
            # ones. The width-128 slice starting at GMAX-1-k has its
            # column k all-ones and every other column zero.
            T = const.tile([128, 128 + GMAX - 1], f16)
            nc.vector.memset(T[:], 0.0)
            nc.vector.memset(T[:, GMAX - 1 : GMAX], 1.0)

            # Per-head persistent state while its groups are in flight.
            hstate = {}

            def load_head(h):
                # Input loads ride the gpsimd queue so the latency-
                # critical r broadcasts on the sync queue never sit
                # behind a 0.5MB head load. Head 0 is the warmup
                # critical path: split its loads into column slices
                # across three queues so the first MM1s can start as
                # soon as their slices land.
                qt_t = heads.tile([D, S], f16, tag="qt")
                kt_t = heads.tile([D, S], f16, tag="kt")
                v_t = heads.tile([BS, NB * BS], f16, tag="v")
                if h == 0:
                    # qt FIRST: the first chunk's MM1s read query runs
                    # scattered across the whole sequence, so they gate
                    # on the LAST qt slice; kt is consumed in ascending
                    # j order so kt slice 0 suffices to start. v is only
                    # needed by the first MM2s (~15us in).
                    engs = [nc.sync, nc.gpsimd]
                    n = 0
                    for t_, d_ in ((qt_t, qt_d), (kt_t, kt_d), (v_t, v_d)):
                        for c in range(4):
                            sl = slice(c * CH, (c + 1) * CH)
                            engs[n % 2].dma_start(
                                out=t_[:, sl], in_=d_[h][:, sl]
                            )
                            n += 1
                else:
                    nc.gpsimd.dma_start(out=qt_t[:], in_=qt_d[h])
                    nc.gpsimd.dma_start(out=kt_t[:], in_=kt_d[h])
                    nc.gpsimd.dma_start(out=v_t[:], in_=v_d[h])
                return (qt_t, kt_t, v_t)

            # Flat list of group work items across heads.
            work = []  # (h, g_idx, c0, ng)
            for h in range(HPC):
                for g_idx, (c0, ng) in enumerate(grps):
                    work.append((h, g_idx, c0, ng))

            inflight = {}  # slot -> dict with tiles for phase C

            def phase_a_b(slot, h, g_idx, c0, ng):
                st = hstate[h]
                qt_t, kt_t, v_t = st["io"]
                e_grp = epool.tile([BS, GMAX * CH], f16, tag="e")
                d_grp = ps_d.tile([128, CH], f32, tag="d")

                def emit_denom(k):
                    # Emitted one chunk late so the PE does not head-of-
                    # line block on exp(k) right after MM1(k).
                    nc.tensor.matmul(
                        d_grp[:, :CH],
                        lhsT=T[:, GMAX - 1 - k : GMAX - 1 - k + 128],
                        rhs=e_grp[:, k * CH : (k + 1) * CH],
                        start=(k == 0),
                        stop=(k == ng - 1),
                    )

                for k in range(ng):
                    used, mm1s, _ = sched[c0 + k]
                    s_ps = ps_s.tile([BS, CH], f32, tag="s")
                    for idx, (off, qoffs, w, j) in enumerate(mm1s):
                        if len(qoffs) == 2:
                            base = qt_t[:, qoffs[0] : qoffs[0] + BS]
                            rhs = bass.AP(
                                tensor=base.tensor,
                                offset=base.offset,
                                ap=[
                                    base.ap[0],
                                    [qoffs[1] - qoffs[0], 2],
                                    [1, BS],
                                ],
                            )
                        else:
                            rhs = qt_t[:, qoffs[0] : qoffs[0] + w]
                        nc.tensor.matmul(
                            s_ps[:, off : off + w],
                            lhsT=kt_t[:, j * BS : (j + 1) * BS],
                            rhs=rhs,
                            start=(idx == 0),
                            stop=(idx == len(mm1s) - 1),
                        )
                    eb = k * CH
                    nc.scalar.activation(
                        e_grp[:, eb : eb + used], s_ps[:, :used], AF.Exp,
                        scale=SCALE,
                    )
                    # Pad partial chunks so every denominator in rows
                    # [:ng] is positive (recip of 0 would trip the
                    # non-finite guard) and the start=True matmul covers
                    # the full bank.
                    if used < CH:
                        nc.gpsimd.memset(e_grp[:, eb + used : eb + CH], 1.0)
                    if k >= 1:
                        emit_denom(k - 1)
                emit_denom(ng - 1)
                # Phase B: batched reciprocal -> f16 -> DRAM -> broadcast.
                r_f32 = rsmpool.tile([GMAX, CH], f32, tag="rf32")
                nc.vector.reciprocal_approx_fast(
                    r_f32[:ng, :], d_grp[:ng, :]
                )
                r_f16 = rsmpool.tile([GMAX, CH], f16, tag="rf16")
                nc.vector.tensor_scalar_mul(r_f16[:ng, :], r_f32[:ng, :], 1.0)
                roff = (h % 2) * rcols_per_head + c0 * CH
                nc.gpsimd.dma_start(
                    out=bass.AP(
                        tensor=r_dram.tensor, offset=roff, ap=[[1, ng * CH]]
                    ),
                    in_=r_f16[:ng, :],
                )
                r_rep = rreppool.tile([BS, GMAX * CH], f16, tag="rr")
                # The sync queue is dedicated to these latency-critical
                # broadcast reads. Split into the same halves the
                # normalize uses: the first-half tensor_tensor only
                # depends (via subtile deps) on the first transfer, so
                # phase C starts earlier, and two in-flight DMAs raise
                # the effective broadcast bandwidth.
                hcols_b = ((ng + 1) // 2) * CH
                for lo, hi in ((0, hcols_b), (hcols_b, ng * CH)):
                    if hi > lo:
                        nc.sync.dma_start(
                            out=r_rep[:, lo:hi],
                            in_=bass.AP(
                                tensor=r_dram.tensor,
                                offset=roff + lo,
                                ap=[[0, 128], [1, hi - lo]],
                            ),
                        )
                inflight[slot] = {
                    "h": h, "c0": c0, "ng": ng,
                    "e": e_grp, "rr": r_rep, "v": v_t,
                }

            def flush_drains(st, h):
                for bk in st["pending_drains"]:
                    o_sb = outpool.tile([D, CH], f16, tag="osb")
                    nc.vector.tensor_scalar_mul(
                        o_sb[:], st["o_ps"][:, bk * CH : (bk + 1) * CH], 1.0
                    )
                    nc.gpsimd.dma_start(
                        out=ot_d[h, :, bk * CH : (bk + 1) * CH], in_=o_sb[:]
                    )
                st["pending_drains"] = []

            def phase_c(slot):
                w = inflight.pop(slot, None)
                if w is None:
                    return
                h = w["h"]
                st = hstate[h]
                flush_drains(st, h)
                if not st["zeroed"]:
                    # Zero never-touched query blocks here (not at head
                    # start) so the memsets are emitted AFTER the
                    # previous head's drains on the same DVE queue.
                    for i in empty_rows:
                        nc.vector.memset(
                            st["o_ps"][:, i * BS : (i + 1) * BS], 0.0
                        )
                    st["zeroed"] = True
                e_grp, r_rep, v_t = w["e"], w["rr"], w["v"]
                ng, c0 = w["ng"], w["c0"]
                eh_grp = ehpool.tile([BS, GMAX * CH], f16, tag="eh")
                # Two-half normalize so the first MM2s start sooner.
                hcols = ((ng + 1) // 2) * CH
                for lo, hi in ((0, hcols), (hcols, ng * CH)):
                    if hi > lo:
                        nc.vector.tensor_tensor(
                            out=eh_grp[:, lo:hi],
                            in0=e_grp[:, lo:hi],
                            in1=r_rep[:, lo:hi],
                            op=mybir.AluOpType.mult,
                        )
                for k in range(ng):
                    _, _, pieces = sched[c0 + k]
                    for qo, wp, op, j in pieces:
                        bk = qo // CH
                        first = bk not in st["started"]
                        st["started"].add(bk)
                        st["remaining"][bk] -= 1
                        done = st["remaining"][bk] == 0
                        nc.tensor.matmul(
                            st["o_ps"][:, qo : qo + wp],
                            lhsT=v_t[:, j * BS : (j + 1) * BS],
                            rhs=eh_grp[:, k * CH + op : k * CH + op + wp],
                            start=first,
                            stop=done,
                        )
                        if done:
                            # Defer the drain one slot so its PSUM read
                            # does not block the DVE wait queue while
                            # this bank's MM2s are still in flight.
                            st["pending_drains"].append(bk)
                st["left"] -= 1
                if st["left"] == 0:
                    # Banks no piece ever touched (fully masked-off
                    # output) drain the empty-row memset zeros.
                    for bk in range(S // CH):
                        if bank_counts[bk] == 0:
                            st["pending_drains"].append(bk)
                    flush_drains(st, h)
                    del hstate[h]

            LAG = 4
            nwork = len(work)
            c_next = 0  # next phase-C slot to consume, in order
            for slot, (h, g_idx, c0, ng) in enumerate(work):
                # Progressively shorter phase-C lag near the end of the
                # work list (4 -> 2) so the pipeline drains during the
                # last head's phase A/B instead of entirely after it.
                # Lag 2 still gives the r DRAM round-trip a full A/B
                # slot (~5us) of cover.
                lag = min(LAG, max(2, nwork - 1 - slot))
                while c_next <= slot - lag:
                    phase_c(c_next)
                    c_next += 1
                if g_idx == 0:
                    o_ps = ps_o.tile([D, S], f32, tag="o")
                    hstate[h] = {
                        "io": load_head(h),
                        "o_ps": o_ps,
                        "started": set(),
                        "remaining": list(bank_counts),
                        "left": len(grps),
                        "zeroed": False,
                        "pending_drains": [],
                    }
                phase_a_b(slot, h, g_idx, c0, ng)
            while c_next < nwork:
                phase_c(c_next)
                c_next += 1

    nc.finalize()
    return nc
_CACHE = {}


def _get_program(mask):
    key = np.asarray(mask).astype(bool).tobytes()
    if key not in _CACHE:
        _CACHE[key] = _build(mask)
    return _CACHE[key]


def _shard_inputs(query, key, value):
    q = np.ascontiguousarray(query, dtype=np.float32).reshape(N_HEADS, S, D)
    k = np.ascontiguousarray(key, dtype=np.float32).reshape(N_HEADS, S, D)
    v = np.ascontiguousarray(value, dtype=np.float32).reshape(N_HEADS, S, D)
    qt = np.ascontiguousarray(q.transpose(0, 2, 1).astype(np.float16))
    kt = np.ascontiguousarray(k.transpose(0, 2, 1).astype(np.float16))
    v16 = np.ascontiguousarray(
        v.reshape(N_HEADS, NB, BS, D).transpose(0, 2, 1, 3).astype(np.float16)
    ).reshape(N_HEADS, BS, NB * BS)
    in_maps = []
    for c in range(N_CORES):
        sl = slice(c * HPC, (c + 1) * HPC)
        in_maps.append(
            {
                "qt": np.ascontiguousarray(qt[sl]),
                "kt": np.ascontiguousarray(kt[sl]),
                "v": np.ascontiguousarray(v16[sl]),
            }
        )
    return in_maps


def _unshard_output(results):
    ot = np.concatenate([r["ot"] for r in results], axis=0)  # (32, D, S) f16
    out = ot.transpose(0, 2, 1).reshape(B, H, S, D)
    return np.ascontiguousarray(out, dtype=np.float32)


def kernel(query, key, value, block_mask, block_size, _trace=False):
    from concourse.bass_utils import run_bass_kernel_spmd

    assert int(block_size) == BS
    nc = _get_program(block_mask)
    in_maps = _shard_inputs(query, key, value)
    res = run_bass_kernel_spmd(nc, in_maps, core_ids=list(range(N_CORES)), trace=_trace)
    out = _unshard_output(res.results)
    if _trace:
        return out, res
    return out

